# revision 18
# baseline (speedup 1.0000x reference)
"""Trainium2 Bass kernel for the 25-bit SNN division iteration.

Math: the reference does a bit-serial two's-complement subtract
R_trial = R - D over 25 LSB-first bit-planes (ripple carry), then
Q = carry_out and R_next = carry_out ? R_trial : R.

Instead of 25 sequential full-adder steps per row, each row's 25 bits are
packed into two exact fp32 integers (lo = bits 0..12, hi = bits 13..24) with
Horner trees, the subtract/borrow/mux runs on the packed values (width = rows,
not rows*bits), and the result is unpacked with fused (mod, is_ge)
tensor_scalar ops.  Everything is exact in fp32 (|values| <= 16383).

Sharding: trivially data-parallel over the batch dim; each of the 8 cores
gets a contiguous block of N/8 rows.
"""

import numpy as np

import concourse.bass as bass
import concourse.mybir as mybir
from concourse.bacc import Bacc
from concourse.tile import TileContext
from concourse.bass_utils import run_bass_kernel_spmd

N = 2097152
BITS = 25
N_CORES = 8
ROWS = N // N_CORES  # 262144 rows per core
P = 128

F32 = mybir.dt.float32
Alu = mybir.AluOpType


def build(K=256, T=8, reps=1, loop_n=0, internal_io=False):
    """Build the per-core Bass module. rows handled = P*K*T.

    reps>1 (python-unrolled) or loop_n>0 (hardware For_i loop) repeat the
    whole compute loop over the same I/O — used only by the timing harness
    to measure per-iteration HW time via the slope method.

    internal_io=True replaces the big external I/O tensors with on-device
    internal DRAM (contents irrelevant) so timing calls don't pay host
    transfer costs; a tiny passthrough keeps the PJRT plumbing happy.
    """
    rows = P * K * T
    nc = Bacc()

    if internal_io:
        R_ext = nc.dram_tensor("R", [rows, BITS], F32)
        D_ext = nc.dram_tensor("D", [rows, BITS], F32)
        Q_ext = nc.dram_tensor("Q", [rows, 1], F32)
        RN_ext = nc.dram_tensor("R_next", [rows, BITS], F32)
        dum_in = nc.dram_tensor("dummy_in", [P, 8], F32, kind="ExternalInput")
        dum_out = nc.dram_tensor("dummy_out", [P, 8], F32, kind="ExternalOutput")
    else:
        R_ext = nc.dram_tensor("R", [rows, BITS], F32, kind="ExternalInput")
        D_ext = nc.dram_tensor("D", [rows, BITS], F32, kind="ExternalInput")
        Q_ext = nc.dram_tensor("Q", [rows, 1], F32, kind="ExternalOutput")
        RN_ext = nc.dram_tensor("R_next", [rows, BITS], F32, kind="ExternalOutput")

    # Contiguous per-partition layout: partition p of tile t holds rows
    # [t*P*K + p*K, t*P*K + (p+1)*K), i.e. K*25 consecutive floats.
    Rv = R_ext[:].rearrange("(t p k) b -> t p (k b)", t=T, p=P, k=K)
    Dv = D_ext[:].rearrange("(t p k) b -> t p (k b)", t=T, p=P, k=K)
    RNv = RN_ext[:].rearrange("(t p k) b -> t p (k b)", t=T, p=P, k=K)
    Qv = Q_ext[:].rearrange("(t p k) one -> p t (k one)", t=T, p=P, k=K)

    v = nc.vector
    gp = nc.gpsimd
    I32 = mybir.dt.int32

    with TileContext(nc) as tc:
        with (
            tc.tile_pool(name="io", bufs=2) as io,
            tc.tile_pool(name="aux", bufs=2) as aux,
            tc.tile_pool(name="fp", bufs=2) as fpool,
            tc.tile_pool(name="qp", bufs=1) as qp,
        ):
            q_tile = qp.tile([P, T * K], F32)

            def rep_body():
              for t in range(T):
                r = io.tile([P, K * BITS], F32, tag="r")
                d = io.tile([P, K * BITS], F32, tag="d")
                nc.sync.dma_start(out=r[:], in_=Rv[t])
                nc.sync.dma_start(out=d[:], in_=Dv[t])

                rb = r[:].rearrange("p (k b) -> p k b", b=BITS)
                db = d[:].rearrange("p (k b) -> p k b", b=BITS)
                # output bits overwrite the R tile (R is dead after packing)
                ob = rb

                r_lo = aux.tile([P, K], F32, tag="r_lo")
                r_hi = aux.tile([P, K], F32, tag="r_hi")
                d_lo = aux.tile([P, K], F32, tag="d_lo")
                d_hi = aux.tile([P, K], F32, tag="d_hi")

                # Pack trees: lo = sum_{i<13} 2^i b_i, hi = sum_{i>=13} 2^(i-13) b_i
                # R lo+hi and D lo on DVE (scalar_tensor_tensor Horner steps);
                # D hi on gpsimd (tensor_scalar + tensor_add pairs).
                def pack_dve(dst, src, lsb, nbits):
                    v.scalar_tensor_tensor(
                        dst[:], src[:, :, lsb + 1], 2.0, src[:, :, lsb],
                        Alu.mult, Alu.add,
                    )
                    for i in range(2, nbits):
                        v.scalar_tensor_tensor(
                            dst[:], src[:, :, lsb + i], float(2**i), dst[:],
                            Alu.mult, Alu.add,
                        )

                def pack_gp(dst, tmp, src, lsb, nbits):
                    gp.tensor_scalar(tmp[:], src[:, :, lsb + 1], 2.0, None, Alu.mult)
                    gp.tensor_add(dst[:], tmp[:], src[:, :, lsb])
                    for i in range(2, nbits):
                        gp.tensor_scalar(
                            tmp[:], src[:, :, lsb + i], float(2**i), None, Alu.mult
                        )
                        gp.tensor_add(dst[:], dst[:], tmp[:])

                gp_tmp = aux.tile([P, K], F32, tag="gp_tmp")
                pack_dve(r_lo, rb, 0, 13)
                pack_dve(r_hi, rb, 13, 12)
                pack_gp(d_lo, gp_tmp, db, 0, 13)
                pack_gp(d_hi, gp_tmp, db, 13, 12)

                u_lo = aux.tile([P, K], F32, tag="u_lo")
                u_hi = aux.tile([P, K], F32, tag="u_hi")
                ncb_lo = aux.tile([P, K], F32, tag="ncb_lo")
                tp = aux.tile([P, K], F32, tag="tp")
                ncb = aux.tile([P, K], F32, tag="ncb")
                ncb_i = aux.tile([P, K], I32, tag="ncb_i")
                s_lo = aux.tile([P, K], F32, tag="s_lo")
                s_hi = aux.tile([P, K], F32, tag="s_hi")

                v.tensor_sub(u_lo[:], r_lo[:], d_lo[:])
                v.tensor_sub(u_hi[:], r_hi[:], d_hi[:])
                # borrow of low half: u_lo < 0
                v.tensor_scalar(ncb_lo[:], u_lo[:], 0.0, None, Alu.is_lt)
                # tp = u_hi - ncb_lo; carry_out = tp >= 0
                v.scalar_tensor_tensor(tp[:], ncb_lo[:], -1.0, u_hi[:], Alu.mult, Alu.add)
                v.tensor_scalar(
                    q_tile[:, t * K:(t + 1) * K], tp[:], 0.0, None, Alu.is_ge
                )
                v.tensor_scalar(ncb[:], tp[:], 0.0, None, Alu.is_lt)
                v.tensor_scalar(ncb_i[:], tp[:], 0.0, None, Alu.is_lt)
                # S_lo = u_lo + 8192*ncb_lo ; S_hi = tp + 4096*ncb
                v.scalar_tensor_tensor(s_lo[:], ncb_lo[:], 8192.0, u_lo[:], Alu.mult, Alu.add)
                v.scalar_tensor_tensor(s_hi[:], ncb[:], 4096.0, tp[:], Alu.mult, Alu.add)
                # mux: where borrow (ncb), keep original R
                v.copy_predicated(s_lo[:], ncb_i[:], r_lo[:])
                v.copy_predicated(s_hi[:], ncb_i[:], r_hi[:])

                # Unpack via exact floor-shifts: f_i = floor(S / 2^i), computed
                # on ACT as rne((S + 0.5 - 2^(i-1)) * 2^-i) -> int32 (tie-free,
                # HW-verified RNE).  bit_i = f_i - 2*f_{i+1}.
                def floors(src, nbits, tag):
                    fs = []
                    for i in range(1, nbits):
                        f = fpool.tile([P, K], I32, tag=f"{tag}{i}")
                        nc.scalar.activation(
                            f[:], src[:],
                            mybir.ActivationFunctionType.Copy,
                            bias=float(2.0 ** (-i - 1) - 0.5),
                            scale=float(2.0 ** -i),
                        )
                        fs.append(f)
                    return fs

                f_lo = floors(s_lo, 13, "flo")   # f_1..f_12
                f_hi = floors(s_hi, 12, "fhi")   # f_1..f_11

                # combines on DVE: bit_i = f_i - 2*f_{i+1}
                def combines(fs, src, nbits, bit_base):
                    v.scalar_tensor_tensor(
                        ob[:, :, bit_base], fs[0][:], -2.0, src[:], Alu.mult, Alu.add
                    )
                    for i in range(1, nbits - 1):
                        v.scalar_tensor_tensor(
                            ob[:, :, bit_base + i], fs[i][:], -2.0, fs[i - 1][:],
                            Alu.mult, Alu.add,
                        )
                    v.tensor_copy(ob[:, :, bit_base + nbits - 1], fs[nbits - 2][:])

                combines(f_lo, s_lo, 13, 0)
                combines(f_hi, s_hi, 12, 13)

                nc.scalar.dma_start(out=RNv[t], in_=r[:])

            if loop_n:
                with tc.For_i(0, loop_n, 1):
                    rep_body()
            else:
                for _rep in range(reps):
                    rep_body()

            nc.sync.dma_start(out=Qv, in_=q_tile[:].rearrange("p (t k) -> p t k", t=T))

            if internal_io:
                dt = io.tile([P, 8], F32, tag="dum")
                nc.sync.dma_start(out=dt[:], in_=dum_in[:])
                nc.sync.dma_start(out=dum_out[:], in_=dt[:])

    nc.finalize()
    return nc


_nc_full = None


def _get_full_nc():
    global _nc_full
    if _nc_full is None:
        _nc_full = build(K=256, T=8)
    return _nc_full


def _run(R, D, trace=False):
    R = np.ascontiguousarray(np.asarray(R, dtype=np.float32))
    D = np.ascontiguousarray(np.asarray(D, dtype=np.float32))
    assert R.shape == (N, BITS) and D.shape == (N, BITS)
    nc = _get_full_nc()
    in_maps = [
        {"R": R[c * ROWS:(c + 1) * ROWS], "D": D[c * ROWS:(c + 1) * ROWS]}
        for c in range(N_CORES)
    ]
    res = run_bass_kernel_spmd(nc, in_maps, list(range(N_CORES)), trace=trace)
    Q = np.concatenate([res.results[c]["Q"] for c in range(N_CORES)], axis=0)
    RN = np.concatenate([res.results[c]["R_next"] for c in range(N_CORES)], axis=0)
    return (Q, RN), res


def kernel(R, D):
    out, _ = _run(R, D, trace=False)
    return out


# revision 19
# speedup vs baseline: 2.6740x; 2.6740x over previous
"""Trainium2 Bass kernel for the 25-bit SNN division iteration.

Math: the reference does a bit-serial two's-complement subtract
R_trial = R - D over 25 LSB-first bit-planes (ripple carry), then
Q = carry_out and R_next = carry_out ? R_trial : R.

Instead of 25 sequential full-adder steps per row, each row's 25 bits are
packed into two exact fp32 integers (lo = bits 0..12, hi = bits 13..24) with
Horner trees, the subtract/borrow/mux runs on the packed values (width = rows,
not rows*bits), and the result is unpacked with fused (mod, is_ge)
tensor_scalar ops.  Everything is exact in fp32 (|values| <= 16383).

Sharding: trivially data-parallel over the batch dim; each of the 8 cores
gets a contiguous block of N/8 rows.
"""

import numpy as np

import concourse.bass as bass
import concourse.mybir as mybir
from concourse.bacc import Bacc
from concourse.tile import TileContext
from concourse.bass_utils import run_bass_kernel_spmd

N = 2097152
BITS = 25
N_CORES = 8
ROWS = N // N_CORES  # 262144 rows per core
P = 128

F32 = mybir.dt.float32
Alu = mybir.AluOpType


def build(K=256, T=8, reps=1, loop_n=0, internal_io=False):
    """Build the per-core Bass module. rows handled = P*K*T.

    reps>1 (python-unrolled) or loop_n>0 (hardware For_i loop) repeat the
    whole compute loop over the same I/O — used only by the timing harness
    to measure per-iteration HW time via the slope method.

    internal_io=True replaces the big external I/O tensors with on-device
    internal DRAM (contents irrelevant) so timing calls don't pay host
    transfer costs; a tiny passthrough keeps the PJRT plumbing happy.
    """
    rows = P * K * T
    nc = Bacc()

    if internal_io:
        R_ext = nc.dram_tensor("R", [rows, BITS], F32)
        D_ext = nc.dram_tensor("D", [rows, BITS], F32)
        Q_ext = nc.dram_tensor("Q", [rows, 1], F32)
        RN_ext = nc.dram_tensor("R_next", [rows, BITS], F32)
        dum_in = nc.dram_tensor("dummy_in", [P, 8], F32, kind="ExternalInput")
        dum_out = nc.dram_tensor("dummy_out", [P, 8], F32, kind="ExternalOutput")
    else:
        R_ext = nc.dram_tensor("R", [rows, BITS], F32, kind="ExternalInput")
        D_ext = nc.dram_tensor("D", [rows, BITS], F32, kind="ExternalInput")
        Q_ext = nc.dram_tensor("Q", [rows, 1], F32, kind="ExternalOutput")
        RN_ext = nc.dram_tensor("R_next", [rows, BITS], F32, kind="ExternalOutput")

    # Contiguous per-partition layout: partition p of tile t holds rows
    # [t*P*K + p*K, t*P*K + (p+1)*K), i.e. K*25 consecutive floats.
    Rv = R_ext[:].rearrange("(t p k) b -> t p (k b)", t=T, p=P, k=K)
    Dv = D_ext[:].rearrange("(t p k) b -> t p (k b)", t=T, p=P, k=K)
    RNv = RN_ext[:].rearrange("(t p k) b -> t p (k b)", t=T, p=P, k=K)
    Qv = Q_ext[:].rearrange("(t p k) one -> p t (k one)", t=T, p=P, k=K)

    v = nc.vector
    gp = nc.gpsimd
    I32 = mybir.dt.int32

    with TileContext(nc) as tc:
        with (
            tc.tile_pool(name="io", bufs=2) as io,
            tc.tile_pool(name="aux", bufs=2) as aux,
            tc.tile_pool(name="fp", bufs=2) as fpool,
            tc.tile_pool(name="qp", bufs=1) as qp,
        ):
            q_tile = qp.tile([P, T * K], F32)

            def rep_body():
              for t in range(T):
                r = io.tile([P, K * BITS], F32, tag="r")
                d = io.tile([P, K * BITS], F32, tag="d")
                nc.sync.dma_start(out=r[:], in_=Rv[t])
                nc.sync.dma_start(out=d[:], in_=Dv[t])

                rb = r[:].rearrange("p (k b) -> p k b", b=BITS)
                db = d[:].rearrange("p (k b) -> p k b", b=BITS)
                # output bits overwrite the R tile (R is dead after packing)
                ob = rb

                r_lo = aux.tile([P, K], F32, tag="r_lo")
                r_hi = aux.tile([P, K], F32, tag="r_hi")
                d_lo = aux.tile([P, K], F32, tag="d_lo")
                d_hi = aux.tile([P, K], F32, tag="d_hi")

                # Pack trees: lo = sum_{i<13} 2^i b_i, hi = sum_{i>=13} 2^(i-13) b_i
                # R lo+hi and D lo on DVE (scalar_tensor_tensor Horner steps);
                # D hi on gpsimd (tensor_scalar + tensor_add pairs).
                def pack_dve(dst, src, lsb, nbits):
                    v.scalar_tensor_tensor(
                        dst[:], src[:, :, lsb + 1], 2.0, src[:, :, lsb],
                        Alu.mult, Alu.add,
                    )
                    for i in range(2, nbits):
                        v.scalar_tensor_tensor(
                            dst[:], src[:, :, lsb + i], float(2**i), dst[:],
                            Alu.mult, Alu.add,
                        )

                def pack_gp(dst, tmp, src, lsb, nbits):
                    gp.tensor_scalar(tmp[:], src[:, :, lsb + 1], 2.0, None, Alu.mult)
                    gp.tensor_add(dst[:], tmp[:], src[:, :, lsb])
                    for i in range(2, nbits):
                        gp.tensor_scalar(
                            tmp[:], src[:, :, lsb + i], float(2**i), None, Alu.mult
                        )
                        gp.tensor_add(dst[:], dst[:], tmp[:])

                pack_dve(r_lo, rb, 0, 13)
                pack_dve(r_hi, rb, 13, 12)
                pack_dve(d_lo, db, 0, 13)
                pack_dve(d_hi, db, 13, 12)

                u_lo = aux.tile([P, K], F32, tag="u_lo")
                u_hi = aux.tile([P, K], F32, tag="u_hi")
                ncb_lo = aux.tile([P, K], F32, tag="ncb_lo")
                tp = aux.tile([P, K], F32, tag="tp")
                ncb = aux.tile([P, K], F32, tag="ncb")
                ncb_i = aux.tile([P, K], I32, tag="ncb_i")
                s_lo = aux.tile([P, K], F32, tag="s_lo")
                s_hi = aux.tile([P, K], F32, tag="s_hi")

                v.tensor_sub(u_lo[:], r_lo[:], d_lo[:])
                v.tensor_sub(u_hi[:], r_hi[:], d_hi[:])
                # borrow of low half: u_lo < 0
                v.tensor_scalar(ncb_lo[:], u_lo[:], 0.0, None, Alu.is_lt)
                # tp = u_hi - ncb_lo; carry_out = tp >= 0
                v.scalar_tensor_tensor(tp[:], ncb_lo[:], -1.0, u_hi[:], Alu.mult, Alu.add)
                v.tensor_scalar(
                    q_tile[:, t * K:(t + 1) * K], tp[:], 0.0, None, Alu.is_ge
                )
                v.tensor_scalar(ncb[:], tp[:], 0.0, None, Alu.is_lt)
                v.tensor_scalar(ncb_i[:], tp[:], 0.0, None, Alu.is_lt)
                # S_lo = u_lo + 8192*ncb_lo ; S_hi = tp + 4096*ncb
                v.scalar_tensor_tensor(s_lo[:], ncb_lo[:], 8192.0, u_lo[:], Alu.mult, Alu.add)
                v.scalar_tensor_tensor(s_hi[:], ncb[:], 4096.0, tp[:], Alu.mult, Alu.add)
                # mux: where borrow (ncb), keep original R
                v.copy_predicated(s_lo[:], ncb_i[:], r_lo[:])
                v.copy_predicated(s_hi[:], ncb_i[:], r_hi[:])

                # Unpack via exact floor-shifts: f_i = floor(S / 2^i), computed
                # on ACT as rne((S + 0.5 - 2^(i-1)) * 2^-i) -> int32 (tie-free,
                # HW-verified RNE).  bit_i = f_i - 2*f_{i+1}.
                def floors(src, nbits, tag):
                    fs = []
                    for i in range(1, nbits):
                        f = fpool.tile([P, K], I32, tag=f"{tag}{i}")
                        nc.scalar.activation(
                            f[:], src[:],
                            mybir.ActivationFunctionType.Copy,
                            bias=float(2.0 ** (-i - 1) - 0.5),
                            scale=float(2.0 ** -i),
                        )
                        fs.append(f)
                    return fs

                f_lo = floors(s_lo, 13, "flo")   # f_1..f_12
                f_hi = floors(s_hi, 12, "fhi")   # f_1..f_11

                # combines on DVE: bit_i = f_i - 2*f_{i+1}
                def combines(fs, src, nbits, bit_base):
                    v.scalar_tensor_tensor(
                        ob[:, :, bit_base], fs[0][:], -2.0, src[:], Alu.mult, Alu.add
                    )
                    for i in range(1, nbits - 1):
                        v.scalar_tensor_tensor(
                            ob[:, :, bit_base + i], fs[i][:], -2.0, fs[i - 1][:],
                            Alu.mult, Alu.add,
                        )
                    v.tensor_copy(ob[:, :, bit_base + nbits - 1], fs[nbits - 2][:])

                combines(f_lo, s_lo, 13, 0)
                combines(f_hi, s_hi, 12, 13)

                nc.scalar.dma_start(out=RNv[t], in_=r[:])

            if loop_n:
                with tc.For_i(0, loop_n, 1):
                    rep_body()
            else:
                for _rep in range(reps):
                    rep_body()

            nc.sync.dma_start(out=Qv, in_=q_tile[:].rearrange("p (t k) -> p t k", t=T))

            if internal_io:
                dt = io.tile([P, 8], F32, tag="dum")
                nc.sync.dma_start(out=dt[:], in_=dum_in[:])
                nc.sync.dma_start(out=dum_out[:], in_=dt[:])

    nc.finalize()
    return nc


_nc_full = None


def _get_full_nc():
    global _nc_full
    if _nc_full is None:
        _nc_full = build(K=256, T=8)
    return _nc_full


def _run(R, D, trace=False):
    R = np.ascontiguousarray(np.asarray(R, dtype=np.float32))
    D = np.ascontiguousarray(np.asarray(D, dtype=np.float32))
    assert R.shape == (N, BITS) and D.shape == (N, BITS)
    nc = _get_full_nc()
    in_maps = [
        {"R": R[c * ROWS:(c + 1) * ROWS], "D": D[c * ROWS:(c + 1) * ROWS]}
        for c in range(N_CORES)
    ]
    res = run_bass_kernel_spmd(nc, in_maps, list(range(N_CORES)), trace=trace)
    Q = np.concatenate([res.results[c]["Q"] for c in range(N_CORES)], axis=0)
    RN = np.concatenate([res.results[c]["R_next"] for c in range(N_CORES)], axis=0)
    return (Q, RN), res


def kernel(R, D):
    out, _ = _run(R, D, trace=False)
    return out


# revision 24
# speedup vs baseline: 3.0509x; 1.1409x over previous
"""Trainium2 Bass kernel for the 25-bit SNN division iteration.

Math: the reference does a bit-serial two's-complement subtract
R_trial = R - D over 25 LSB-first bit-planes (ripple carry), then
Q = carry_out and R_next = carry_out ? R_trial : R.

Instead of 25 sequential full-adder steps per row, each row's 25 bits are
packed into two exact fp32 integers (lo = bits 0..12, hi = bits 13..24) with
Horner trees, the subtract/borrow/mux runs on the packed values (width = rows,
not rows*bits), and the result is unpacked with fused (mod, is_ge)
tensor_scalar ops.  Everything is exact in fp32 (|values| <= 16383).

Sharding: trivially data-parallel over the batch dim; each of the 8 cores
gets a contiguous block of N/8 rows.
"""

import numpy as np

import concourse.bass as bass
import concourse.mybir as mybir
from concourse.bacc import Bacc
from concourse.tile import TileContext
from concourse.bass_utils import run_bass_kernel_spmd

N = 2097152
BITS = 25
N_CORES = 8
ROWS = N // N_CORES  # 262144 rows per core
P = 128

F32 = mybir.dt.float32
Alu = mybir.AluOpType


def build(K=256, T=8, reps=1, loop_n=0, internal_io=False, variant="full"):
    """Build the per-core Bass module. rows handled = P*K*T.

    reps>1 (python-unrolled) or loop_n>0 (hardware For_i loop) repeat the
    whole compute loop over the same I/O — used only by the timing harness
    to measure per-iteration HW time via the slope method.

    internal_io=True replaces the big external I/O tensors with on-device
    internal DRAM (contents irrelevant) so timing calls don't pay host
    transfer costs; a tiny passthrough keeps the PJRT plumbing happy.
    """
    rows = P * K * T
    nc = Bacc()

    if internal_io:
        R_ext = nc.dram_tensor("R", [rows, BITS], F32)
        D_ext = nc.dram_tensor("D", [rows, BITS], F32)
        Q_ext = nc.dram_tensor("Q", [rows, 1], F32)
        RN_ext = nc.dram_tensor("R_next", [rows, BITS], F32)
        dum_in = nc.dram_tensor("dummy_in", [P, 8], F32, kind="ExternalInput")
        dum_out = nc.dram_tensor("dummy_out", [P, 8], F32, kind="ExternalOutput")
    else:
        R_ext = nc.dram_tensor("R", [rows, BITS], F32, kind="ExternalInput")
        D_ext = nc.dram_tensor("D", [rows, BITS], F32, kind="ExternalInput")
        Q_ext = nc.dram_tensor("Q", [rows, 1], F32, kind="ExternalOutput")
        RN_ext = nc.dram_tensor("R_next", [rows, BITS], F32, kind="ExternalOutput")

    # Contiguous per-partition layout: partition p of tile t holds rows
    # [t*P*K + p*K, t*P*K + (p+1)*K), i.e. K*25 consecutive floats.
    Rv = R_ext[:].rearrange("(t p k) b -> t p (k b)", t=T, p=P, k=K)
    Dv = D_ext[:].rearrange("(t p k) b -> t p (k b)", t=T, p=P, k=K)
    RNv = RN_ext[:].rearrange("(t p k) b -> t p (k b)", t=T, p=P, k=K)
    Qv = Q_ext[:].rearrange("(t p k) one -> p t (k one)", t=T, p=P, k=K)

    v = nc.vector
    gp = nc.gpsimd
    I32 = mybir.dt.int32

    with TileContext(nc) as tc:
        with (
            tc.tile_pool(name="io", bufs=2) as io,
            tc.tile_pool(name="aux", bufs=2) as aux,
            tc.tile_pool(name="fp", bufs=2) as fpool,
            tc.tile_pool(name="qp", bufs=1) as qp,
        ):
            q_tile = qp.tile([P, T * K], F32)

            if variant == "compute":
                # persistent tiles loaded once; loop is pure compute
                r_perm = io.tile([P, K * BITS], F32, tag="r")
                d_perm = io.tile([P, K * BITS], F32, tag="d")
                nc.sync.dma_start(out=r_perm[:], in_=Rv[0])
                nc.sync.dma_start(out=d_perm[:], in_=Dv[0])

            def rep_body():
              for t in range(T):
                if variant == "compute":
                    r, d = r_perm, d_perm
                else:
                    r = io.tile([P, K * BITS], F32, tag="r")
                    d = io.tile([P, K * BITS], F32, tag="d")
                    nc.sync.dma_start(out=r[:], in_=Rv[t])
                    nc.sync.dma_start(out=d[:], in_=Dv[t])
                if variant == "dma":
                    nc.scalar.dma_start(out=RNv[t], in_=d[:])
                    continue

                rb = r[:].rearrange("p (k b) -> p k b", b=BITS)
                db = d[:].rearrange("p (k b) -> p k b", b=BITS)
                # output bits overwrite the R tile (R is dead after packing)
                ob = rb

                r_lo = aux.tile([P, K], F32, tag="r_lo")
                r_hi = aux.tile([P, K], F32, tag="r_hi")
                d_lo = aux.tile([P, K], F32, tag="d_lo")
                d_hi = aux.tile([P, K], F32, tag="d_hi")

                # Pack trees: lo = sum_{i<13} 2^i b_i, hi = sum_{i>=13} 2^(i-13) b_i
                # R lo+hi and D lo on DVE (scalar_tensor_tensor Horner steps);
                # D hi on gpsimd (tensor_scalar + tensor_add pairs).
                def pack_dve(dst, src, lsb, nbits):
                    v.scalar_tensor_tensor(
                        dst[:], src[:, :, lsb + 1], 2.0, src[:, :, lsb],
                        Alu.mult, Alu.add,
                    )
                    for i in range(2, nbits):
                        v.scalar_tensor_tensor(
                            dst[:], src[:, :, lsb + i], float(2**i), dst[:],
                            Alu.mult, Alu.add,
                        )

                def pack_gp(dst, tmp, src, lsb, nbits):
                    gp.tensor_scalar(tmp[:], src[:, :, lsb + 1], 2.0, None, Alu.mult)
                    gp.tensor_add(dst[:], tmp[:], src[:, :, lsb])
                    for i in range(2, nbits):
                        gp.tensor_scalar(
                            tmp[:], src[:, :, lsb + i], float(2**i), None, Alu.mult
                        )
                        gp.tensor_add(dst[:], dst[:], tmp[:])

                pack_dve(r_lo, rb, 0, 13)
                pack_dve(r_hi, rb, 13, 12)
                pack_dve(d_lo, db, 0, 13)
                pack_dve(d_hi, db, 13, 12)

                u_lo = aux.tile([P, K], F32, tag="u_lo")
                u_hi = aux.tile([P, K], F32, tag="u_hi")
                ncb_lo = aux.tile([P, K], F32, tag="ncb_lo")
                tp = aux.tile([P, K], F32, tag="tp")
                ncb = aux.tile([P, K], F32, tag="ncb")
                ncb_i = aux.tile([P, K], I32, tag="ncb_i")
                s_lo = aux.tile([P, K], F32, tag="s_lo")
                s_hi = aux.tile([P, K], F32, tag="s_hi")

                v.tensor_sub(u_lo[:], r_lo[:], d_lo[:])
                v.tensor_sub(u_hi[:], r_hi[:], d_hi[:])
                # borrow of low half: u_lo < 0
                v.tensor_scalar(ncb_lo[:], u_lo[:], 0.0, None, Alu.is_lt)
                # tp = u_hi - ncb_lo; carry_out = tp >= 0
                v.scalar_tensor_tensor(tp[:], ncb_lo[:], -1.0, u_hi[:], Alu.mult, Alu.add)
                v.tensor_scalar(
                    q_tile[:, t * K:(t + 1) * K], tp[:], 0.0, None, Alu.is_ge
                )
                v.tensor_scalar(ncb[:], tp[:], 0.0, None, Alu.is_lt)
                v.tensor_scalar(ncb_i[:], tp[:], 0.0, None, Alu.is_lt)
                # S_lo = u_lo + 8192*ncb_lo ; S_hi = tp + 4096*ncb
                v.scalar_tensor_tensor(s_lo[:], ncb_lo[:], 8192.0, u_lo[:], Alu.mult, Alu.add)
                v.scalar_tensor_tensor(s_hi[:], ncb[:], 4096.0, tp[:], Alu.mult, Alu.add)
                # mux: where borrow (ncb), keep original R
                v.copy_predicated(s_lo[:], ncb_i[:], r_lo[:])
                v.copy_predicated(s_hi[:], ncb_i[:], r_hi[:])

                # Unpack via exact floor-shifts: f_i = floor(S / 2^i), computed
                # on ACT as rne((S + 0.5 - 2^(i-1)) * 2^-i) -> int32 (tie-free,
                # HW-verified RNE).  bit_i = f_i - 2*f_{i+1}.
                def floors(src, nbits, tag):
                    fs = []
                    for i in range(1, nbits):
                        f = fpool.tile([P, K], I32, tag=f"{tag}{i}")
                        nc.scalar.activation(
                            f[:], src[:],
                            mybir.ActivationFunctionType.Copy,
                            bias=float(2.0 ** (-i - 1) - 0.5),
                            scale=float(2.0 ** -i),
                        )
                        fs.append(f)
                    return fs

                f_lo = floors(s_lo, 13, "flo")   # f_1..f_12
                f_hi = floors(s_hi, 12, "fhi")   # f_1..f_11

                # combines on DVE: bit_i = f_i - 2*f_{i+1}
                def combines(fs, src, nbits, bit_base):
                    v.scalar_tensor_tensor(
                        ob[:, :, bit_base], fs[0][:], -2.0, src[:], Alu.mult, Alu.add
                    )
                    for i in range(1, nbits - 1):
                        v.scalar_tensor_tensor(
                            ob[:, :, bit_base + i], fs[i][:], -2.0, fs[i - 1][:],
                            Alu.mult, Alu.add,
                        )
                    v.tensor_copy(ob[:, :, bit_base + nbits - 1], fs[nbits - 2][:])

                combines(f_lo, s_lo, 13, 0)
                combines(f_hi, s_hi, 12, 13)

                if variant != "compute":
                    nc.scalar.dma_start(out=RNv[t], in_=r[:])

            if loop_n:
                with tc.For_i(0, loop_n, 1):
                    rep_body()
            else:
                for _rep in range(reps):
                    rep_body()

            if variant != "dma":
                nc.sync.dma_start(
                    out=Qv, in_=q_tile[:].rearrange("p (t k) -> p t k", t=T)
                )

            if internal_io:
                dt = io.tile([P, 8], F32, tag="dum")
                nc.sync.dma_start(out=dt[:], in_=dum_in[:])
                nc.sync.dma_start(out=dum_out[:], in_=dt[:])

    nc.finalize()
    return nc


_nc_full = None


def _get_full_nc():
    global _nc_full
    if _nc_full is None:
        _nc_full = build(K=256, T=8)
    return _nc_full


def _run(R, D, trace=False):
    R = np.ascontiguousarray(np.asarray(R, dtype=np.float32))
    D = np.ascontiguousarray(np.asarray(D, dtype=np.float32))
    assert R.shape == (N, BITS) and D.shape == (N, BITS)
    nc = _get_full_nc()
    in_maps = [
        {"R": R[c * ROWS:(c + 1) * ROWS], "D": D[c * ROWS:(c + 1) * ROWS]}
        for c in range(N_CORES)
    ]
    res = run_bass_kernel_spmd(nc, in_maps, list(range(N_CORES)), trace=trace)
    Q = np.concatenate([res.results[c]["Q"] for c in range(N_CORES)], axis=0)
    RN = np.concatenate([res.results[c]["R_next"] for c in range(N_CORES)], axis=0)
    return (Q, RN), res


def kernel(R, D):
    out, _ = _run(R, D, trace=False)
    return out


# revision 26
# speedup vs baseline: 3.4933x; 1.1450x over previous
"""Trainium2 Bass kernel for the 25-bit SNN division iteration.

Math: the reference does a bit-serial two's-complement subtract
R_trial = R - D over 25 LSB-first bit-planes (ripple carry), then
Q = carry_out and R_next = carry_out ? R_trial : R.

Approach (all exact in fp32; values <= 16383):
  1. Pack each row's 25 bits into two integers (lo = bits 0..12,
     hi = bits 13..24) with a level-wise binary tree: each tree level is a
     single wide scalar_tensor_tensor op over 3D access patterns
     (pairs -> quads -> octets -> halves), 9 DVE ops per tensor.
  2. Borrow/select logic on the packed values; comparisons are computed on
     the Scalar(ACT) engine via Sign (sign(2x+1) maps >=0 to +1).
  3. Mux (keep R on borrow) via one copy_predicated with an int32 mask.
  4. Unpack: floor-shifts f_i = floor(S/2^i) on ACT as
     rne((S + 0.5 - 2^(i-1)) * 2^-i) -> int16 (HW converts with RNE;
     tie-free because the fraction is strictly inside (-0.5, 0.5)), all
     f_i landing in one tile so that bit_i = f_i - 2*f_{i+1} collapses
     into one batched stt per half.
Sharding: batch dim split evenly across the 8 cores; no communication.
"""

import numpy as np

import bass_rust
import concourse.bass as bass
import concourse.mybir as mybir
from concourse.bacc import Bacc
from concourse.tile import TileContext
from concourse.bass_utils import run_bass_kernel_spmd

N = 2097152
BITS = 25
N_CORES = 8
ROWS = N // N_CORES  # 262144 rows per core
P = 128

F32 = mybir.dt.float32
I16 = mybir.dt.int16
I32 = mybir.dt.int32
Alu = mybir.AluOpType
Act = mybir.ActivationFunctionType


def _ap(tile_ap, offset, dims):
    """Raw AP on a tile's tensor: dims = [[step, count], ...] incl partition."""
    return bass_rust.AP(tile_ap.tensor, offset, dims)


def build(K=256, T=8, reps=1, loop_n=0, internal_io=False, variant="full"):
    """Build the per-core Bass module. rows handled = P*K*T.

    reps / loop_n repeat the compute loop over the same I/O (timing only).
    internal_io=True uses on-device DRAM for the big tensors (timing only).
    variant: "full" | "dma" (loads+stores only) | "compute" (no big DMAs).
    """
    rows = P * K * T
    nc = Bacc()

    if internal_io:
        R_ext = nc.dram_tensor("R", [rows, BITS], F32)
        D_ext = nc.dram_tensor("D", [rows, BITS], F32)
        Q_ext = nc.dram_tensor("Q", [rows, 1], F32)
        RN_ext = nc.dram_tensor("R_next", [rows, BITS], F32)
        dum_in = nc.dram_tensor("dummy_in", [P, 8], F32, kind="ExternalInput")
        dum_out = nc.dram_tensor("dummy_out", [P, 8], F32, kind="ExternalOutput")
    else:
        R_ext = nc.dram_tensor("R", [rows, BITS], F32, kind="ExternalInput")
        D_ext = nc.dram_tensor("D", [rows, BITS], F32, kind="ExternalInput")
        Q_ext = nc.dram_tensor("Q", [rows, 1], F32, kind="ExternalOutput")
        RN_ext = nc.dram_tensor("R_next", [rows, BITS], F32, kind="ExternalOutput")

    Rv = R_ext[:].rearrange("(t p k) b -> t p (k b)", t=T, p=P, k=K)
    Dv = D_ext[:].rearrange("(t p k) b -> t p (k b)", t=T, p=P, k=K)
    RNv = RN_ext[:].rearrange("(t p k) b -> t p (k b)", t=T, p=P, k=K)
    Qv = Q_ext[:].rearrange("(t p k) one -> p t (k one)", t=T, p=P, k=K)

    v = nc.vector
    sc = nc.scalar
    W = K * BITS

    with TileContext(nc) as tc:
        with (
            tc.tile_pool(name="io", bufs=2) as io,
            tc.tile_pool(name="pk", bufs=1) as pk,
            tc.tile_pool(name="aux", bufs=2) as aux,
            tc.tile_pool(name="fp", bufs=2) as fpool,
            tc.tile_pool(name="qp", bufs=1) as qp,
        ):
            q_tile = qp.tile([P, T * K], F32)

            if variant == "compute":
                r_perm = io.tile([P, W], F32, tag="r")
                d_perm = pk.tile([P, W], F32, tag="d")
                nc.sync.dma_start(out=r_perm[:], in_=Rv[0])
                nc.sync.dma_start(out=d_perm[:], in_=Dv[0])

            def pack(src, dst, tag):
                """25 LSB-first bit-planes (per 25-elem group) -> packed
                lo (bits 0..12) at dst[:, 0:K], hi (bits 13..24) at
                dst[:, K:2K].  Level-wise tree, one stt per level+half."""
                w1l = pk.tile([P, 6 * K], F32, tag=f"{tag}w1l")
                w1h = pk.tile([P, 6 * K], F32, tag=f"{tag}w1h")
                w2l = pk.tile([P, 3 * K], F32, tag=f"{tag}w2l")
                w2h = pk.tile([P, 3 * K], F32, tag=f"{tag}w2h")
                ol = pk.tile([P, K], F32, tag=f"{tag}ol")
                oh = pk.tile([P, K], F32, tag=f"{tag}oh")
                tl = pk.tile([P, K], F32, tag=f"{tag}tl")
                s = src[:]
                # L1: pairs w_j = b_{2j} + 2*b_{2j+1}
                v.scalar_tensor_tensor(
                    w1l[:].rearrange("p (k j) -> p k j", j=6),
                    _ap(s, 1, [[W, P], [BITS, K], [2, 6]]), 2.0,
                    _ap(s, 0, [[W, P], [BITS, K], [2, 6]]),
                    Alu.mult, Alu.add)
                v.scalar_tensor_tensor(
                    w1h[:].rearrange("p (k j) -> p k j", j=6),
                    _ap(s, 14, [[W, P], [BITS, K], [2, 6]]), 2.0,
                    _ap(s, 13, [[W, P], [BITS, K], [2, 6]]),
                    Alu.mult, Alu.add)
                # L2: quads q_j = w_{2j} + 4*w_{2j+1}
                for wt, qt in ((w1l, w2l), (w1h, w2h)):
                    v.scalar_tensor_tensor(
                        qt[:].rearrange("p (k j) -> p k j", j=3),
                        _ap(wt[:], 1, [[6 * K, P], [6, K], [2, 3]]), 4.0,
                        _ap(wt[:], 0, [[6 * K, P], [6, K], [2, 3]]),
                        Alu.mult, Alu.add)
                # L3: octets o = q_0 + 16*q_1
                for qt, ot in ((w2l, ol), (w2h, oh)):
                    v.scalar_tensor_tensor(
                        ot[:],
                        _ap(qt[:], 1, [[3 * K, P], [3, K]]), 16.0,
                        _ap(qt[:], 0, [[3 * K, P], [3, K]]),
                        Alu.mult, Alu.add)
                # L4: lo = o + 256*q_2 + 4096*b12 ; hi = o + 256*q_2
                v.scalar_tensor_tensor(
                    tl[:], _ap(w2l[:], 2, [[3 * K, P], [3, K]]), 256.0,
                    ol[:], Alu.mult, Alu.add)
                v.scalar_tensor_tensor(
                    dst[:, 0:K], _ap(s, 12, [[W, P], [BITS, K]]), 4096.0,
                    tl[:], Alu.mult, Alu.add)
                v.scalar_tensor_tensor(
                    dst[:, K:2 * K], _ap(w2h[:], 2, [[3 * K, P], [3, K]]),
                    256.0, oh[:], Alu.mult, Alu.add)

            def rep_body():
              for t in range(T):
                if variant == "compute":
                    r, d = r_perm, d_perm
                else:
                    r = io.tile([P, W], F32, tag="r")
                    d = pk.tile([P, W], F32, tag="d")
                    nc.sync.dma_start(out=r[:], in_=Rv[t])
                    nc.sync.dma_start(out=d[:], in_=Dv[t])
                if variant == "dma":
                    nc.scalar.dma_start(out=RNv[t], in_=d[:])
                    continue

                rp = aux.tile([P, 2 * K], I16, tag="rp")
                dp = aux.tile([P, 2 * K], I16, tag="dp")
                pack(r, rp, "r")
                pack(d, dp, "d")

                u = aux.tile([P, 2 * K], F32, tag="u")
                v.tensor_sub(u[:], rp[:], dp[:])  # [u_lo | u_hi]

                sgl = aux.tile([P, K], F32, tag="sgl")
                ncb_lo = aux.tile([P, K], F32, tag="ncb_lo")
                tp = aux.tile([P, K], F32, tag="tp")
                sgh = aux.tile([P, K], F32, tag="sgh")
                ncb = aux.tile([P, K], F32, tag="ncb")
                ncb_i = aux.tile([P, 2 * K], I16, tag="ncb_i")

                # borrow_lo = (u_lo < 0) via sign(2*u_lo + 1)
                sc.activation(sgl[:], u[:, 0:K], Act.Sign, bias=1.0, scale=2.0)
                sc.activation(ncb_lo[:], sgl[:], Act.Copy, bias=0.5, scale=-0.5)
                # tp = u_hi - borrow_lo ; carry_out = (tp >= 0)
                v.scalar_tensor_tensor(
                    tp[:], ncb_lo[:], -1.0, u[:, K:2 * K], Alu.mult, Alu.add)
                sc.activation(sgh[:], tp[:], Act.Sign, bias=1.0, scale=2.0)
                # Q = carry_out
                sc.activation(
                    q_tile[:, t * K:(t + 1) * K], sgh[:], Act.Copy,
                    bias=0.5, scale=0.5)
                sc.activation(ncb[:], sgh[:], Act.Copy, bias=0.5, scale=-0.5)
                sc.activation(ncb_i[:, 0:K], sgh[:], Act.Copy, bias=0.5, scale=-0.5)
                sc.activation(ncb_i[:, K:2 * K], sgh[:], Act.Copy, bias=0.5, scale=-0.5)

                # F holds S then the floor-shifts: lo half slots 0..12 (13K),
                # hi half slots 13..24 (12K); int16 (values <= 8191).
                F = fpool.tile([P, BITS * K], I16, tag="F")
                # S_lo = u_lo + 8192*borrow_lo ; S_hi = tp + 4096*(1-carry)
                v.scalar_tensor_tensor(
                    F[:, 0:K], ncb_lo[:], 8192.0, u[:, 0:K], Alu.mult, Alu.add)
                v.scalar_tensor_tensor(
                    F[:, 13 * K:14 * K], ncb[:], 4096.0, tp[:], Alu.mult, Alu.add)
                # mux: where borrow, replace S by packed R (both halves, 1 op)
                v.copy_predicated(
                    _ap(F[:], 0, [[BITS * K, P], [13 * K, 2], [1, K]]),
                    ncb_i[:].rearrange("p (h k) -> p h k", h=2),
                    rp[:].rearrange("p (h k) -> p h k", h=2))

                # floor-shifts on ACT: f_i = rne((S + 0.5 - 2^(i-1)) * 2^-i)
                for i in range(1, 13):
                    sc.activation(
                        F[:, i * K:(i + 1) * K], F[:, 0:K], Act.Copy,
                        bias=float(2.0 ** (-i - 1) - 0.5), scale=float(2.0 ** -i))
                for i in range(1, 12):
                    sc.activation(
                        F[:, (13 + i) * K:(14 + i) * K], F[:, 13 * K:14 * K],
                        Act.Copy,
                        bias=float(2.0 ** (-i - 1) - 0.5), scale=float(2.0 ** -i))

                # combines: bit_i = f_i - 2*f_{i+1}, one stt per half
                ob = r[:]  # output bits overwrite the R tile
                v.scalar_tensor_tensor(
                    _ap(ob, 0, [[W, P], [1, 12], [BITS, K]]),
                    _ap(F[:], K, [[BITS * K, P], [K, 12], [1, K]]), -2.0,
                    _ap(F[:], 0, [[BITS * K, P], [K, 12], [1, K]]),
                    Alu.mult, Alu.add)
                v.scalar_tensor_tensor(
                    _ap(ob, 13, [[W, P], [1, 11], [BITS, K]]),
                    _ap(F[:], 14 * K, [[BITS * K, P], [K, 11], [1, K]]), -2.0,
                    _ap(F[:], 13 * K, [[BITS * K, P], [K, 11], [1, K]]),
                    Alu.mult, Alu.add)
                # top bits are the last floors (0/1) -> strided copies on ACT
                sc.activation(
                    _ap(ob, 12, [[W, P], [BITS, K]]), F[:, 12 * K:13 * K],
                    Act.Copy, bias=0.0, scale=1.0)
                sc.activation(
                    _ap(ob, 24, [[W, P], [BITS, K]]), F[:, 24 * K:25 * K],
                    Act.Copy, bias=0.0, scale=1.0)

                if variant != "compute":
                    nc.scalar.dma_start(out=RNv[t], in_=r[:])

            if loop_n:
                with tc.For_i(0, loop_n, 1):
                    rep_body()
            else:
                for _rep in range(reps):
                    rep_body()

            if variant != "dma":
                nc.sync.dma_start(
                    out=Qv, in_=q_tile[:].rearrange("p (t k) -> p t k", t=T)
                )

            if internal_io:
                dt = io.tile([P, 8], F32, tag="dum")
                nc.sync.dma_start(out=dt[:], in_=dum_in[:])
                nc.sync.dma_start(out=dum_out[:], in_=dt[:])

    nc.finalize()
    return nc


_nc_full = None


def _get_full_nc():
    global _nc_full
    if _nc_full is None:
        _nc_full = build(K=256, T=8)
    return _nc_full


def _run(R, D, trace=False):
    R = np.ascontiguousarray(np.asarray(R, dtype=np.float32))
    D = np.ascontiguousarray(np.asarray(D, dtype=np.float32))
    assert R.shape == (N, BITS) and D.shape == (N, BITS)
    nc = _get_full_nc()
    in_maps = [
        {"R": R[c * ROWS:(c + 1) * ROWS], "D": D[c * ROWS:(c + 1) * ROWS]}
        for c in range(N_CORES)
    ]
    res = run_bass_kernel_spmd(nc, in_maps, list(range(N_CORES)), trace=trace)
    Q = np.concatenate([res.results[c]["Q"] for c in range(N_CORES)], axis=0)
    RN = np.concatenate([res.results[c]["R_next"] for c in range(N_CORES)], axis=0)
    return (Q, RN), res


def kernel(R, D):
    out, _ = _run(R, D, trace=False)
    return out


# revision 27
# speedup vs baseline: 3.9776x; 1.1387x over previous
"""Trainium2 Bass kernel for the 25-bit SNN division iteration.

Math: the reference does a bit-serial two's-complement subtract
R_trial = R - D over 25 LSB-first bit-planes (ripple carry), then
Q = carry_out and R_next = carry_out ? R_trial : R.

Approach (all exact in fp32; values <= 16383):
  1. Pack each row's 25 bits into two integers (lo = bits 0..12,
     hi = bits 13..24) with a level-wise binary tree: each tree level is a
     single wide scalar_tensor_tensor op over 3D access patterns
     (pairs -> quads -> octets -> halves), 9 DVE ops per tensor.
  2. Borrow/select logic on the packed values; comparisons are computed on
     the Scalar(ACT) engine via Sign (sign(2x+1) maps >=0 to +1).
  3. Mux (keep R on borrow) via one copy_predicated with an int32 mask.
  4. Unpack: floor-shifts f_i = floor(S/2^i) on ACT as
     rne((S + 0.5 - 2^(i-1)) * 2^-i) -> int16 (HW converts with RNE;
     tie-free because the fraction is strictly inside (-0.5, 0.5)), all
     f_i landing in one tile so that bit_i = f_i - 2*f_{i+1} collapses
     into one batched stt per half.
Sharding: batch dim split evenly across the 8 cores; no communication.
"""

import numpy as np

import bass_rust
import concourse.bass as bass
import concourse.mybir as mybir
from concourse.bacc import Bacc
from concourse.tile import TileContext
from concourse.bass_utils import run_bass_kernel_spmd

N = 2097152
BITS = 25
N_CORES = 8
ROWS = N // N_CORES  # 262144 rows per core
P = 128

F32 = mybir.dt.float32
I16 = mybir.dt.int16
I32 = mybir.dt.int32
Alu = mybir.AluOpType
Act = mybir.ActivationFunctionType


def _ap(tile_ap, offset, dims):
    """Raw AP on a tile's tensor: dims = [[step, count], ...] incl partition."""
    return bass_rust.AP(tile_ap.tensor, offset, dims)


def build(K=256, T=8, reps=1, loop_n=0, internal_io=False, variant="full"):
    """Build the per-core Bass module. rows handled = P*K*T.

    reps / loop_n repeat the compute loop over the same I/O (timing only).
    internal_io=True uses on-device DRAM for the big tensors (timing only).
    variant: "full" | "dma" (loads+stores only) | "compute" (no big DMAs).
    """
    rows = P * K * T
    nc = Bacc()

    if internal_io:
        R_ext = nc.dram_tensor("R", [rows, BITS], F32)
        D_ext = nc.dram_tensor("D", [rows, BITS], F32)
        Q_ext = nc.dram_tensor("Q", [rows, 1], F32)
        RN_ext = nc.dram_tensor("R_next", [rows, BITS], F32)
        dum_in = nc.dram_tensor("dummy_in", [P, 8], F32, kind="ExternalInput")
        dum_out = nc.dram_tensor("dummy_out", [P, 8], F32, kind="ExternalOutput")
    else:
        R_ext = nc.dram_tensor("R", [rows, BITS], F32, kind="ExternalInput")
        D_ext = nc.dram_tensor("D", [rows, BITS], F32, kind="ExternalInput")
        Q_ext = nc.dram_tensor("Q", [rows, 1], F32, kind="ExternalOutput")
        RN_ext = nc.dram_tensor("R_next", [rows, BITS], F32, kind="ExternalOutput")

    Rv = R_ext[:].rearrange("(t p k) b -> t p (k b)", t=T, p=P, k=K)
    Dv = D_ext[:].rearrange("(t p k) b -> t p (k b)", t=T, p=P, k=K)
    RNv = RN_ext[:].rearrange("(t p k) b -> t p (k b)", t=T, p=P, k=K)
    Qv = Q_ext[:].rearrange("(t p k) one -> p t (k one)", t=T, p=P, k=K)

    v = nc.vector
    sc = nc.scalar
    W = K * BITS

    with TileContext(nc) as tc:
        with (
            tc.tile_pool(name="io", bufs=2) as io,
            tc.tile_pool(name="pk", bufs=1) as pk,
            tc.tile_pool(name="aux", bufs=3) as aux,
            tc.tile_pool(name="fp", bufs=2) as fpool,
            tc.tile_pool(name="qp", bufs=1) as qp,
        ):
            q_tile = qp.tile([P, T * K], F32)

            if variant == "compute":
                r_perm = io.tile([P, W], F32, tag="r")
                d_perm = pk.tile([P, W], F32, tag="d")
                nc.sync.dma_start(out=r_perm[:], in_=Rv[0])
                nc.sync.dma_start(out=d_perm[:], in_=Dv[0])

            def pack(src, dst, tag):
                """25 LSB-first bit-planes (per 25-elem group) -> packed
                lo (bits 0..12) at dst[:, 0:K], hi (bits 13..24) at
                dst[:, K:2K].  Level-wise tree, one stt per level+half."""
                w1l = pk.tile([P, 6 * K], F32, tag=f"{tag}w1l")
                w1h = pk.tile([P, 6 * K], F32, tag=f"{tag}w1h")
                w2l = pk.tile([P, 3 * K], F32, tag=f"{tag}w2l")
                w2h = pk.tile([P, 3 * K], F32, tag=f"{tag}w2h")
                ol = pk.tile([P, K], F32, tag=f"{tag}ol")
                oh = pk.tile([P, K], F32, tag=f"{tag}oh")
                tl = pk.tile([P, K], F32, tag=f"{tag}tl")
                s = src[:]
                # L1: pairs w_j = b_{2j} + 2*b_{2j+1}
                v.scalar_tensor_tensor(
                    w1l[:].rearrange("p (k j) -> p k j", j=6),
                    _ap(s, 1, [[W, P], [BITS, K], [2, 6]]), 2.0,
                    _ap(s, 0, [[W, P], [BITS, K], [2, 6]]),
                    Alu.mult, Alu.add)
                v.scalar_tensor_tensor(
                    w1h[:].rearrange("p (k j) -> p k j", j=6),
                    _ap(s, 14, [[W, P], [BITS, K], [2, 6]]), 2.0,
                    _ap(s, 13, [[W, P], [BITS, K], [2, 6]]),
                    Alu.mult, Alu.add)
                # L2: quads q_j = w_{2j} + 4*w_{2j+1}
                for wt, qt in ((w1l, w2l), (w1h, w2h)):
                    v.scalar_tensor_tensor(
                        qt[:].rearrange("p (k j) -> p k j", j=3),
                        _ap(wt[:], 1, [[6 * K, P], [6, K], [2, 3]]), 4.0,
                        _ap(wt[:], 0, [[6 * K, P], [6, K], [2, 3]]),
                        Alu.mult, Alu.add)
                # L3: octets o = q_0 + 16*q_1
                for qt, ot in ((w2l, ol), (w2h, oh)):
                    v.scalar_tensor_tensor(
                        ot[:],
                        _ap(qt[:], 1, [[3 * K, P], [3, K]]), 16.0,
                        _ap(qt[:], 0, [[3 * K, P], [3, K]]),
                        Alu.mult, Alu.add)
                # L4: lo = o + 256*q_2 + 4096*b12 ; hi = o + 256*q_2
                v.scalar_tensor_tensor(
                    tl[:], _ap(w2l[:], 2, [[3 * K, P], [3, K]]), 256.0,
                    ol[:], Alu.mult, Alu.add)
                v.scalar_tensor_tensor(
                    dst[:, 0:K], _ap(s, 12, [[W, P], [BITS, K]]), 4096.0,
                    tl[:], Alu.mult, Alu.add)
                v.scalar_tensor_tensor(
                    dst[:, K:2 * K], _ap(w2h[:], 2, [[3 * K, P], [3, K]]),
                    256.0, oh[:], Alu.mult, Alu.add)

            def rep_body():
              for t in range(T):
                if variant == "compute":
                    r, d = r_perm, d_perm
                else:
                    r = io.tile([P, W], F32, tag="r")
                    d = pk.tile([P, W], F32, tag="d")
                    nc.sync.dma_start(out=r[:], in_=Rv[t])
                    nc.sync.dma_start(out=d[:], in_=Dv[t])
                if variant == "dma":
                    nc.scalar.dma_start(out=RNv[t], in_=d[:])
                    continue

                rp = aux.tile([P, 2 * K], I16, tag="rp")
                dp = aux.tile([P, 2 * K], I16, tag="dp")
                pack(r, rp, "r")
                pack(d, dp, "d")

                u = aux.tile([P, 2 * K], F32, tag="u")
                v.tensor_sub(u[:], rp[:], dp[:])  # [u_lo | u_hi]

                ncb_lo = aux.tile([P, K], F32, tag="ncb_lo")
                tp = aux.tile([P, K], F32, tag="tp")
                ncb = aux.tile([P, K], F32, tag="ncb")
                ncb_i = aux.tile([P, 2 * K], I16, tag="ncb_i")

                # borrow_lo = (u_lo < 0)
                v.tensor_scalar(ncb_lo[:], u[:, 0:K], 0.0, None, Alu.is_lt)
                # tp = u_hi - borrow_lo ; carry_out = (tp >= 0)
                v.scalar_tensor_tensor(
                    tp[:], ncb_lo[:], -1.0, u[:, K:2 * K], Alu.mult, Alu.add)
                # Q = carry_out
                v.tensor_scalar(
                    q_tile[:, t * K:(t + 1) * K], tp[:], 0.0, None, Alu.is_ge)
                v.tensor_scalar(ncb[:], tp[:], 0.0, None, Alu.is_lt)
                v.tensor_scalar(ncb_i[:, 0:K], tp[:], 0.0, None, Alu.is_lt)
                v.tensor_scalar(ncb_i[:, K:2 * K], tp[:], 0.0, None, Alu.is_lt)

                # F holds S then the floor-shifts: lo half slots 0..12 (13K),
                # hi half slots 13..24 (12K); int16 (values <= 8191).
                F = fpool.tile([P, BITS * K], I16, tag="F")
                # S_lo = u_lo + 8192*borrow_lo ; S_hi = tp + 4096*(1-carry)
                v.scalar_tensor_tensor(
                    F[:, 0:K], ncb_lo[:], 8192.0, u[:, 0:K], Alu.mult, Alu.add)
                v.scalar_tensor_tensor(
                    F[:, 13 * K:14 * K], ncb[:], 4096.0, tp[:], Alu.mult, Alu.add)
                # mux: where borrow, replace S by packed R (both halves, 1 op)
                v.copy_predicated(
                    _ap(F[:], 0, [[BITS * K, P], [13 * K, 2], [1, K]]),
                    ncb_i[:].rearrange("p (h k) -> p h k", h=2),
                    rp[:].rearrange("p (h k) -> p h k", h=2))

                # floor-shifts on ACT: f_i = rne((S + 0.5 - 2^(i-1)) * 2^-i)
                for i in range(1, 13):
                    sc.activation(
                        F[:, i * K:(i + 1) * K], F[:, 0:K], Act.Copy,
                        bias=float(2.0 ** (-i - 1) - 0.5), scale=float(2.0 ** -i))
                for i in range(1, 12):
                    sc.activation(
                        F[:, (13 + i) * K:(14 + i) * K], F[:, 13 * K:14 * K],
                        Act.Copy,
                        bias=float(2.0 ** (-i - 1) - 0.5), scale=float(2.0 ** -i))

                # combines: bit_i = f_i - 2*f_{i+1}, one stt per half
                ob = r[:]  # output bits overwrite the R tile
                v.scalar_tensor_tensor(
                    _ap(ob, 0, [[W, P], [1, 12], [BITS, K]]),
                    _ap(F[:], K, [[BITS * K, P], [K, 12], [1, K]]), -2.0,
                    _ap(F[:], 0, [[BITS * K, P], [K, 12], [1, K]]),
                    Alu.mult, Alu.add)
                v.scalar_tensor_tensor(
                    _ap(ob, 13, [[W, P], [1, 11], [BITS, K]]),
                    _ap(F[:], 14 * K, [[BITS * K, P], [K, 11], [1, K]]), -2.0,
                    _ap(F[:], 13 * K, [[BITS * K, P], [K, 11], [1, K]]),
                    Alu.mult, Alu.add)
                # top bits are the last floors (0/1) -> strided copies on ACT
                sc.activation(
                    _ap(ob, 12, [[W, P], [BITS, K]]), F[:, 12 * K:13 * K],
                    Act.Copy, bias=0.0, scale=1.0)
                sc.activation(
                    _ap(ob, 24, [[W, P], [BITS, K]]), F[:, 24 * K:25 * K],
                    Act.Copy, bias=0.0, scale=1.0)

                if variant != "compute":
                    nc.scalar.dma_start(out=RNv[t], in_=r[:])

            if loop_n:
                with tc.For_i(0, loop_n, 1):
                    rep_body()
            else:
                for _rep in range(reps):
                    rep_body()

            if variant != "dma":
                nc.sync.dma_start(
                    out=Qv, in_=q_tile[:].rearrange("p (t k) -> p t k", t=T)
                )

            if internal_io:
                dt = io.tile([P, 8], F32, tag="dum")
                nc.sync.dma_start(out=dt[:], in_=dum_in[:])
                nc.sync.dma_start(out=dum_out[:], in_=dt[:])

    nc.finalize()
    return nc


_nc_full = None


def _get_full_nc():
    global _nc_full
    if _nc_full is None:
        _nc_full = build(K=256, T=8)
    return _nc_full


def _run(R, D, trace=False):
    R = np.ascontiguousarray(np.asarray(R, dtype=np.float32))
    D = np.ascontiguousarray(np.asarray(D, dtype=np.float32))
    assert R.shape == (N, BITS) and D.shape == (N, BITS)
    nc = _get_full_nc()
    in_maps = [
        {"R": R[c * ROWS:(c + 1) * ROWS], "D": D[c * ROWS:(c + 1) * ROWS]}
        for c in range(N_CORES)
    ]
    res = run_bass_kernel_spmd(nc, in_maps, list(range(N_CORES)), trace=trace)
    Q = np.concatenate([res.results[c]["Q"] for c in range(N_CORES)], axis=0)
    RN = np.concatenate([res.results[c]["R_next"] for c in range(N_CORES)], axis=0)
    return (Q, RN), res


def kernel(R, D):
    out, _ = _run(R, D, trace=False)
    return out
